# revision 1
# baseline (speedup 1.0000x reference)
"""DFMConv2d Trainium2 kernel.

Reference computation (per sample b):
  pooled = mean_{h,w} x[b]                          [C=256]
  h      = relu(pooled @ w1.T + b1)                 [128]
  mix    = softmax((h @ w2.T + b2).reshape(256, 8)) [256, 8]
  y      = conv3x3_SAME(x[b], base_filters)         [8, 64, 64]
  out[b] = einsum('on,nhw->ohw', mix, y)            [256, 64, 64]

Strategy (8 NeuronCores, data-parallel over batch, 8 samples/core), all
heavy matmuls in float32r (~2e-4 rel err):

  conv:  y_tap[(t,n), hw] = sum_c filt[t,n,c] * x[c, hw] — all 9 taps in
         the stationary M dim (M=72), so x streams through the PE exactly
         twice; 16 matmuls/sample into a row-padded flat buffer
         ypad[72, 1+66*64+2] (rows -1 and 64 zeroed).
  shift: z[(t,n), hw] = y_tap shifted by (dy-1, dx-1) — one fully
         CONTIGUOUS SBUF->SBUF DMA per tap (offset dy*64+dx into ypad),
         then 6 tiny column-zero fixups for the dx!=1 wraparound cells.
  mix:   out[o, hw] = mixT72.T @ z with K=72; mixT72 = softmax(mix).T
         replicated 9x via 4 doubling partition-shift DMAs.
  DMA issue is split across rings: x loads on GPSIMD/SWDGE, out stores on
  the ACT HWDGE ring, z/mixT/params on the SP ring — avoids FIFO
  head-of-line blocking between pipeline stages.
"""
import sys

sys.path.insert(0, "/opt/trn_rl_repo")

import numpy as np
import ml_dtypes

import concourse.bass as bass
import concourse.bacc as bacc
import concourse.tile as tile
import concourse.mybir as mybir
from concourse.bass_utils import run_bass_kernel_spmd
from contextlib import ExitStack

F32 = mybir.dt.float32
F32R = mybir.dt.float32r
AFT = mybir.ActivationFunctionType
AXX = mybir.AxisListType.X
ALU = mybir.AluOpType

N_CORES = 8
BPC = 8            # samples per core
C = 256
CO = 256
H = W = 64
HW = H * W
NB = 8             # n_base
HID = 128
CCH = 2            # channel chunks of 128
NHC = 8            # h-chunks (8 output rows each)
NT = 9             # taps
M88 = 88           # taps grouped by dx at 32-aligned bases: rows 32*dx+8*dy..+8
YP_LEN = 1 + 66 * 64 + 2   # lead zero + 66 rows + tail slack (reads reach 4225)
TAP_ROW = {(dy, dx): 32 * dx + 8 * dy for dy in range(3) for dx in range(3)}

_BUILT = None


def _build():
    nc = bacc.Bacc("TRN2", target_bir_lowering=False)

    d_x = nc.dram_tensor("x", [BPC, C, HW], F32R, kind="ExternalInput")
    d_w1t = nc.dram_tensor("w1t", [C, HID], F32, kind="ExternalInput")
    d_b1 = nc.dram_tensor("b1", [HID, 1], F32, kind="ExternalInput")
    d_w2p = nc.dram_tensor("w2p", [HID, NB, CO], F32, kind="ExternalInput")
    d_b2t = nc.dram_tensor("b2t", [128, 2, NB], F32, kind="ExternalInput")
    d_ft = nc.dram_tensor("ft", [128, CCH, M88], F32R, kind="ExternalInput")
    d_id = nc.dram_tensor("ident", [128, 128], F32, kind="ExternalInput")
    d_z0 = nc.dram_tensor("zeros", [128, 66], F32R, kind="ExternalInput")
    d_out = nc.dram_tensor("out", [BPC, 2, 128, HW], F32, kind="ExternalOutput")

    with tile.TileContext(nc) as tc, ExitStack() as ctx:
        prm = ctx.enter_context(tc.tile_pool(name="prm", bufs=1))
        xp = ctx.enter_context(tc.tile_pool(name="xp", bufs=2))
        ypp = ctx.enter_context(tc.tile_pool(name="ypp", bufs=2))
        zp = ctx.enter_context(tc.tile_pool(name="zp", bufs=2))
        op = ctx.enter_context(tc.tile_pool(name="op", bufs=3))
        sm = ctx.enter_context(tc.tile_pool(name="sm", bufs=2))
        ps_c = ctx.enter_context(tc.tile_pool(name="ps_c", bufs=2, space="PSUM"))
        ps_m = ctx.enter_context(tc.tile_pool(name="ps_m", bufs=3, space="PSUM"))
        ps_s = ctx.enter_context(tc.tile_pool(name="ps_s", bufs=2, space="PSUM"))

        # ---- params (loaded once) ----
        w1t_sb = prm.tile([128, CCH, HID], F32, tag="w1t")
        nc.sync.dma_start(out=w1t_sb, in_=d_w1t[:, :].rearrange("(cc p) h -> p cc h", p=128))
        b1_sb = prm.tile([128, 1], F32, tag="b1")
        nc.sync.dma_start(out=b1_sb, in_=d_b1[:, :])
        w2p_sb = prm.tile([HID, NB, CO], F32, tag="w2p")
        nc.sync.dma_start(out=w2p_sb, in_=d_w2p[:, :, :])
        b2t_sb = prm.tile([128, 2, NB], F32, tag="b2t")
        nc.sync.dma_start(out=b2t_sb, in_=d_b2t[:, :, :])
        ft_sb = prm.tile([128, CCH, M88], F32R, tag="ft")
        nc.sync.dma_start(out=ft_sb, in_=d_ft[:, :, :])
        id_sb = prm.tile([128, 128], F32, tag="ident")
        nc.sync.dma_start(out=id_sb, in_=d_id[:, :])
        z0_sb = prm.tile([128, 66], F32R, tag="z0")
        nc.sync.dma_start(out=z0_sb, in_=d_z0[:, :])
        pooled_sb = prm.tile([128, CCH, BPC], F32, tag="pooled")
        h_sb = prm.tile([128, BPC], F32, tag="h")
        trash = prm.tile([128, HW], F32, tag="trash")

        for j in range(BPC):
            # ---- load (SWDGE ring) + pooling (split DVE / ACT-accum) ----
            xt = xp.tile([128, CCH, HW], F32R, tag="x")
            nc.gpsimd.dma_start(
                out=xt, in_=d_x[j, :, :].rearrange("(cc p) hw -> p cc hw", p=128))
            nc.vector.reduce_sum(
                pooled_sb[:, 0, j:j + 1], xt[:, 0, :].bitcast(F32), axis=AXX)
            nc.scalar.activation(out=trash, in_=xt[:, 1, :].bitcast(F32),
                                 func=AFT.Copy, accum_out=pooled_sb[:, 1, j:j + 1])

            # ---- attention MLP (fp32) ----
            ph = ps_s.tile([128, 1], F32, tag="sm")
            for cc in range(CCH):
                nc.tensor.matmul(ph, w1t_sb[:, cc, :], pooled_sb[:, cc, j:j + 1],
                                 start=(cc == 0), stop=(cc == 1))
            nc.scalar.activation(out=h_sb[:, j:j + 1], in_=ph, func=AFT.Relu,
                                 bias=b1_sb, scale=1.0)

            mixT_sb = sm.tile([M88, 2, 128], F32R, tag="mixT")
            for oc in range(2):
                pl = ps_s.tile([128, NB], F32, tag="sm")
                for n in range(NB):
                    nc.tensor.matmul(pl[:, n:n + 1],
                                     w2p_sb[:, n, oc * 128:(oc + 1) * 128],
                                     h_sb[:, j:j + 1], start=True, stop=True)
                lg_sb = sm.tile([128, NB], F32, tag="lg_sb")
                nc.vector.tensor_tensor(out=lg_sb, in0=pl, in1=b2t_sb[:, oc, :],
                                        op=ALU.add)
                ex_sb = sm.tile([128, NB], F32, tag="ex_sb")
                nc.scalar.activation(out=ex_sb, in_=lg_sb, func=AFT.Exp)
                sums = sm.tile([128, 1], F32, tag="sums")
                nc.vector.reduce_sum(sums, ex_sb, axis=AXX)
                rec = sm.tile([128, 1], F32, tag="rec")
                nc.vector.reciprocal(rec, sums)
                mix_sb = sm.tile([128, NB], F32, tag="mix_sb")
                nc.vector.tensor_scalar_mul(out=mix_sb, in0=ex_sb, scalar1=rec)
                ptr = ps_s.tile([NB, 128], F32, tag="sm")
                nc.tensor.transpose(ptr, mix_sb, id_sb)
                # DVE cast fp32 -> f32r counts as a rounding producer
                nc.vector.tensor_copy(mixT_sb[0:NB, oc, :], ptr)
            # replicate rows [0:8) nine times via doubling partition-shift DMAs
            nc.sync.dma_start(out=mixT_sb[8:16], in_=mixT_sb[0:8])
            nc.sync.dma_start(out=mixT_sb[16:32], in_=mixT_sb[0:16])
            nc.sync.dma_start(out=mixT_sb[32:64], in_=mixT_sb[0:32])
            nc.sync.dma_start(out=mixT_sb[64:88], in_=mixT_sb[0:24])

            # ---- conv into row-padded flat y_tap ----
            ypad = ypp.tile([M88, YP_LEN], F32R, tag="ypad")
            nc.vector.tensor_copy(ypad[:, 0:65].bitcast(F32),
                                  z0_sb[0:M88, 0:65].bitcast(F32))
            nc.vector.tensor_copy(ypad[:, 4161:4226].bitcast(F32),
                                  z0_sb[0:M88, 0:65].bitcast(F32))
            for hc in range(NHC):
                yps = ps_c.tile([128, 512], F32, tag="yps")
                for cc in range(CCH):
                    nc.tensor.matmul(yps[0:M88, :], ft_sb[:, cc, :],
                                     xt[:, cc, 512 * hc:512 * (hc + 1)],
                                     start=(cc == 0), stop=(cc == 1))
                nc.scalar.copy(
                    out=ypad[:, 65 + 512 * hc:65 + 512 * (hc + 1)].bitcast(F32),
                    in_=yps[0:M88, :])

            # ---- per-tap shifted windows into z (contiguous DMAs) ----
            zt = zp.tile([M88, HW], F32R, tag="z")
            ztv = zt.rearrange("p (h w) -> p h w", w=64)
            for dy in range(3):
                for dx in range(3):
                    r = TAP_ROW[(dy, dx)]
                    off = dy * 64 + dx
                    # dy=2 taps in dx groups 0,1 also copy the zeroed gap rows
                    # (ypad rows r+8..r+16 are zero via the zero filter cols),
                    # so z has no uninitialized rows under the K=88 contraction
                    nr = 16 if (dy == 2 and dx < 2) else NB
                    nc.sync.dma_start(out=zt[r:r + nr, :],
                                      in_=ypad[r:r + nr, off:off + HW])
            # zero the dx wraparound columns: col 0 for dx=0 (rows 0:24),
            # col 63 for dx=2 (rows 64:88)
            nc.vector.tensor_copy(
                ztv[0:24, :, 0:1].rearrange("p h w -> p (h w)"),
                z0_sb[0:24, 0:64].bitcast(F32))
            nc.vector.tensor_copy(
                ztv[64:88, :, 63:64].rearrange("p h w -> p (h w)"),
                z0_sb[64:88, 0:64].bitcast(F32))

            # ---- mix: out[o, hw] = mixT72.T @ z (K=72, f32r) ----
            for oc in range(2):
                ot = op.tile([128, HW], F32, tag="out")
                for hc in range(NHC):
                    om = ps_m.tile([128, 512], F32, tag="ops")
                    nc.tensor.matmul(om, mixT_sb[:, oc, :],
                                     zt[:, 512 * hc:512 * (hc + 1)],
                                     start=True, stop=True)
                    if hc % 2 == 0:
                        nc.vector.tensor_copy(ot[:, 512 * hc:512 * (hc + 1)], om)
                    else:
                        nc.scalar.copy(out=ot[:, 512 * hc:512 * (hc + 1)], in_=om)
                nc.scalar.dma_start(out=d_out[j, oc, :, :], in_=ot)

    nc.compile()
    return nc


def _prep_inputs(x, w1, b1, w2, b2, base_filters):
    """Host-side input layout prep. Returns per-core in_maps."""
    B = x.shape[0]
    xs = np.ascontiguousarray(x.reshape(B, C, HW)).astype(np.float32)
    w1t = np.ascontiguousarray(w1.T).astype(np.float32) / float(HW)
    b1c = np.ascontiguousarray(b1.reshape(HID, 1)).astype(np.float32)
    w2p = np.ascontiguousarray(w2.reshape(CO, NB, HID).transpose(2, 1, 0)).astype(np.float32)
    b2t = np.ascontiguousarray(b2.reshape(2, 128, NB).transpose(1, 0, 2)).astype(np.float32)
    filt = base_filters.reshape(NB, CCH, 128, 3, 3)  # [n, cc, cp, dy, dx]
    # ft[c_part, cc, 32*dx + 8*dy + n] = filt[n, cc, c_part, dy, dx]; gaps zero
    ft = np.zeros((128, CCH, M88), dtype=np.float32)
    for dy in range(3):
        for dx in range(3):
            r = 32 * dx + 8 * dy
            ft[:, :, r:r + NB] = filt[:, :, :, dy, dx].transpose(2, 1, 0)
    ident = np.eye(128, dtype=np.float32)
    zeros = np.zeros((128, 66), dtype=np.float32)

    in_maps = []
    for core in range(N_CORES):
        in_maps.append({
            "x": np.ascontiguousarray(xs[core * BPC:(core + 1) * BPC]),
            "w1t": w1t, "b1": b1c, "w2p": w2p, "b2t": b2t,
            "ft": ft, "ident": ident, "zeros": zeros,
        })
    return in_maps


def kernel(x, w1, b1, w2, b2, base_filters):
    global _BUILT
    if _BUILT is None:
        _BUILT = _build()
    nc = _BUILT
    in_maps = _prep_inputs(np.asarray(x, dtype=np.float32),
                           np.asarray(w1, dtype=np.float32),
                           np.asarray(b1, dtype=np.float32),
                           np.asarray(w2, dtype=np.float32),
                           np.asarray(b2, dtype=np.float32),
                           np.asarray(base_filters, dtype=np.float32))
    res = run_bass_kernel_spmd(nc, in_maps, core_ids=list(range(N_CORES)))
    outs = []
    for core in range(N_CORES):
        o = res.results[core]["out"]            # [BPC, 2, 128, HW]
        outs.append(o.reshape(BPC, CO, H, W))
    return np.concatenate(outs, axis=0).astype(np.float32)



# revision 11
# speedup vs baseline: 1.1764x; 1.1764x over previous
"""DFMConv2d Trainium2 kernel (bf16 data path).

Reference computation (per sample b):
  pooled = mean_{h,w} x[b]                          [C=256]
  h      = relu(pooled @ w1.T + b1)                 [128]
  mix    = softmax((h @ w2.T + b2).reshape(256, 8)) [256, 8]
  y      = conv3x3_SAME(x[b], base_filters)         [8, 64, 64]
  out[b] = einsum('on,nhw->ohw', mix, y)            [256, 64, 64]

Strategy (8 NeuronCores, data-parallel over batch, 8 samples/core).
All bulk data is bf16 (x, conv filters, conv output, mix weights, out)
— the 2e-2 rel-err budget tolerates it (~5e-3 measured) and it halves
every byte of DMA traffic, which is the bottleneck.  MLP/softmax stay
fp32.

  conv:  y_tap[(t,n), hw] = sum_c filt[t,n,c] * x[c, hw] — all 9 taps in
         the stationary M dim (M=128 padded, 9 taps x 8 bases at chosen
         partition slots), so x streams through the PE exactly twice;
         16 matmuls/sample into a row-padded flat buffer
         ypad[96, 65+64*64+65] (lead/tail halo zeroed).
  shift: z[(t,n), hw] = y_tap shifted by (dy*64+dx) — one contiguous
         SBUF->SBUF DMA per tap.  Tap partition slots are spread over
         partitions 0-95 (dx=0 at 0..23, dx=1 at 24..39+64..71, dx=2 at
         72..95) so the shift descriptors cover all 16 SBUF ports
         instead of piling onto the even ones.  Gap rows carry stale
         finite data and multiply against zero mix weights — never
         copied or zeroed (one-time init).
  mix:   out[o, hw] = mixT.T @ z with K=96; mixT = softmax(mix).T
         replicated to the 9 tap slots via 3 partition-shift DMAs.
  DMA rings: x loads on GPSIMD/SWDGE, out stores on the ACT HWDGE ring,
  z/mixT/params on the SP ring.  PSUM->SBUF copies are spread across
  DVE / ACT / GPSIMD.
"""
import sys

sys.path.insert(0, "/opt/trn_rl_repo")

import numpy as np
import ml_dtypes

import concourse.bass as bass
import concourse.bacc as bacc
import concourse.tile as tile
import concourse.mybir as mybir
from concourse.bass_utils import run_bass_kernel_spmd
from contextlib import ExitStack

F32 = mybir.dt.float32
BF16 = mybir.dt.bfloat16
AFT = mybir.ActivationFunctionType
AXX = mybir.AxisListType.X
ALU = mybir.AluOpType

N_CORES = 8
BPC = 8            # samples per core
C = 256
CO = 256
H = W = 64
HW = H * W
NB = 8             # n_base
HID = 128
CCH = 2            # channel chunks of 128
NHC = 8            # h-chunks (8 output rows each)
M120 = 120         # tap rows live in partitions 0..119 (with gaps)
YP_LEN = 65 + 64 * 64 + 65   # lead halo + 64 rows + tail halo (reads reach 130+4096)
# tap slots: dx=0 contiguous at base 0, dx=2 contiguous at base 96 (the
# column fixups need 32-aligned compute-op bases), dx=1 split so shift
# descriptors land on both even and odd SBUF ports
TAP_ROW = {(0, 0): 0, (1, 0): 8, (2, 0): 16,
           (0, 1): 24, (1, 1): 32, (2, 1): 64,
           (0, 2): 96, (1, 2): 104, (2, 2): 112}

_BUILT = None


def _build():
    nc = bacc.Bacc("TRN2", target_bir_lowering=False)

    d_x = nc.dram_tensor("x", [BPC, 128, CCH, HW], BF16, kind="ExternalInput")
    d_w1t = nc.dram_tensor("w1t", [128, CCH, HID], F32, kind="ExternalInput")
    d_b1 = nc.dram_tensor("b1", [HID, 1], F32, kind="ExternalInput")
    d_w2p = nc.dram_tensor("w2p", [HID, NB, CO], F32, kind="ExternalInput")
    d_b2t = nc.dram_tensor("b2t", [128, 2, NB], F32, kind="ExternalInput")
    d_ft = nc.dram_tensor("ft", [128, CCH, 128], BF16, kind="ExternalInput")
    d_id = nc.dram_tensor("ident", [128, 128], F32, kind="ExternalInput")
    d_z0 = nc.dram_tensor("zeros", [128, 512], BF16, kind="ExternalInput")
    d_out = nc.dram_tensor("out", [BPC, 128, 2, HW], BF16, kind="ExternalOutput")

    with tile.TileContext(nc) as tc, ExitStack() as ctx:
        prm = ctx.enter_context(tc.tile_pool(name="prm", bufs=1))
        xp = ctx.enter_context(tc.tile_pool(name="xp", bufs=2))
        ypp = ctx.enter_context(tc.tile_pool(name="ypp", bufs=2))
        zp = ctx.enter_context(tc.tile_pool(name="zp", bufs=2))
        op = ctx.enter_context(tc.tile_pool(name="op", bufs=3))
        sm = ctx.enter_context(tc.tile_pool(name="sm", bufs=2))
        ps_c = ctx.enter_context(tc.tile_pool(name="ps_c", bufs=2, space="PSUM"))
        ps_m = ctx.enter_context(tc.tile_pool(name="ps_m", bufs=3, space="PSUM"))
        ps_s = ctx.enter_context(tc.tile_pool(name="ps_s", bufs=2, space="PSUM"))

        # ---- params (loaded once) ----
        w1t_sb = prm.tile([128, CCH, HID], F32, tag="w1t")
        nc.sync.dma_start(out=w1t_sb, in_=d_w1t[:, :, :])
        b1_sb = prm.tile([128, 1], F32, tag="b1")
        nc.sync.dma_start(out=b1_sb, in_=d_b1[:, :])
        w2p_sb = prm.tile([HID, NB, CO], F32, tag="w2p")
        nc.sync.dma_start(out=w2p_sb, in_=d_w2p[:, :, :])
        b2t_sb = prm.tile([128, 2, NB], F32, tag="b2t")
        nc.sync.dma_start(out=b2t_sb, in_=d_b2t[:, :, :])
        ft_sb = prm.tile([128, CCH, 128], BF16, tag="ft")
        nc.sync.dma_start(out=ft_sb, in_=d_ft[:, :, :])
        id_sb = prm.tile([128, 128], F32, tag="ident")
        nc.sync.dma_start(out=id_sb, in_=d_id[:, :])
        z0_sb = prm.tile([128, 512], BF16, tag="z0")
        nc.sync.dma_start(out=z0_sb, in_=d_z0[:, :])
        pooled_sb = prm.tile([128, CCH, BPC], F32, tag="pooled")
        h_sb = prm.tile([128, BPC], F32, tag="h")

        def copy_eng(i, out, in_):
            # PSUM->SBUF copies: only DVE and ACT can read PSUM
            if i % 2 == 0:
                nc.vector.tensor_copy(out, in_)
            else:
                nc.scalar.copy(out=out, in_=in_)

        for j in range(BPC):
            # ---- load (SWDGE ring) + pooling (DVE cc0 / GPSIMD cc1) ----
            xt = xp.tile([128, CCH, HW], BF16, tag="x")
            nc.gpsimd.dma_start(out=xt, in_=d_x[j, :, :, :])
            nc.vector.reduce_sum(
                pooled_sb[:, :, j], xt[:, :, :], axis=AXX)

            # ---- attention MLP (fp32) ----
            ph = ps_s.tile([128, 1], F32, tag="sm")
            for cc in range(CCH):
                nc.tensor.matmul(ph, w1t_sb[:, cc, :], pooled_sb[:, cc, j:j + 1],
                                 start=(cc == 0), stop=(cc == 1))
            nc.scalar.activation(out=h_sb[:, j:j + 1], in_=ph, func=AFT.Relu,
                                 bias=b1_sb, scale=1.0)

            mixT_sb = sm.tile([M120, 2, 128], BF16, tag="mixT")
            # zero everything first: gap rows face stale z data and must
            # contribute exactly 0 to the K=120 contraction
            nc.gpsimd.tensor_copy(mixT_sb.rearrange("p a b -> p (a b)"),
                                  z0_sb[0:M120, 0:256])
            for oc in range(2):
                pl = ps_s.tile([128, NB], F32, tag="sm")
                for n in range(NB):
                    nc.tensor.matmul(pl[:, n:n + 1],
                                     w2p_sb[:, n, oc * 128:(oc + 1) * 128],
                                     h_sb[:, j:j + 1], start=True, stop=True)
                lg_sb = sm.tile([128, NB], F32, tag="lg_sb")
                nc.vector.tensor_tensor(out=lg_sb, in0=pl, in1=b2t_sb[:, oc, :],
                                        op=ALU.add)
                ex_sb = sm.tile([128, NB], F32, tag="ex_sb")
                nc.scalar.activation(out=ex_sb, in_=lg_sb, func=AFT.Exp)
                sums = sm.tile([128, 1], F32, tag="sums")
                nc.vector.reduce_sum(sums, ex_sb, axis=AXX)
                rec = sm.tile([128, 1], F32, tag="rec")
                nc.vector.reciprocal(rec, sums)
                mix_sb = sm.tile([128, NB], F32, tag="mix_sb")
                nc.vector.tensor_scalar_mul(out=mix_sb, in0=ex_sb, scalar1=rec)
                ptr = ps_s.tile([NB, 128], F32, tag="sm")
                nc.tensor.transpose(ptr, mix_sb, id_sb)
                nc.vector.tensor_copy(mixT_sb[0:NB, oc, :], ptr)
            # replicate seed rows [0:8) to the 9 tap slots; each DMA's
            # source range must be disjoint from its dest range (no
            # intra-DMA ordering guarantee)
            nc.sync.dma_start(out=mixT_sb[8:16], in_=mixT_sb[0:8])
            nc.sync.dma_start(out=mixT_sb[16:32], in_=mixT_sb[0:16])
            nc.sync.dma_start(out=mixT_sb[32:40], in_=mixT_sb[0:8])
            nc.sync.dma_start(out=mixT_sb[64:72], in_=mixT_sb[0:8])
            nc.sync.dma_start(out=mixT_sb[96:120], in_=mixT_sb[0:24])

            # ---- conv into row-padded flat y_tap ----
            ypad = ypp.tile([M120, YP_LEN], BF16, tag="ypad")
            nc.gpsimd.tensor_copy(ypad[:, 0:65], z0_sb[0:M120, 0:65])
            nc.gpsimd.tensor_copy(ypad[:, 65 + 4096:65 + 4096 + 65],
                                  z0_sb[0:M120, 0:65])
            for hc in range(NHC):
                yps = ps_c.tile([128, 512], F32, tag="yps")
                for cc in range(CCH):
                    nc.tensor.matmul(yps, ft_sb[:, cc, :],
                                     xt[:, cc, 512 * hc:512 * (hc + 1)],
                                     start=(cc == 0), stop=(cc == 1))
                copy_eng(hc, ypad[:, 65 + 512 * hc:65 + 512 * (hc + 1)],
                         yps[0:M120, :])

            # ---- per-tap shifted windows into z (contiguous DMAs) ----
            zt = zp.tile([M120, HW], BF16, tag="z")
            ztv = zt.rearrange("p (h w) -> p h w", w=64)
            if j < 2:
                # one-time finite init of the gap rows (never copied;
                # they multiply against zero mix weights, but must not
                # hold NaN bit patterns from cold SBUF)
                for s in range(8):
                    nc.gpsimd.tensor_copy(zt[32:64, 512 * s:512 * (s + 1)],
                                          z0_sb[32:64, 0:512])
                    nc.gpsimd.tensor_copy(zt[64:96, 512 * s:512 * (s + 1)],
                                          z0_sb[64:96, 0:512])
            for (dy, dx), r in TAP_ROW.items():
                off = dy * 64 + dx
                nc.sync.dma_start(out=zt[r:r + 8, :],
                                  in_=ypad[r:r + 8, off:off + HW])
            # zero the dx wraparound columns: col 0 for dx=0 (rows 0:24),
            # col 63 for dx=2 (rows 72:96)
            nc.gpsimd.tensor_copy(
                ztv[0:24, :, 0:1].rearrange("p h w -> p (h w)"),
                z0_sb[0:24, 0:64])
            nc.gpsimd.tensor_copy(
                ztv[96:120, :, 63:64].rearrange("p h w -> p (h w)"),
                z0_sb[96:120, 0:64])

            # ---- mix: out[o, hw] = mixT.T @ z (K=96, bf16) ----
            ot = op.tile([128, 2, HW], BF16, tag="out")
            for oc in range(2):
                for hc in range(NHC):
                    om = ps_m.tile([128, 512], F32, tag="ops")
                    nc.tensor.matmul(om, mixT_sb[:, oc, :],
                                     zt[:, 512 * hc:512 * (hc + 1)],
                                     start=True, stop=True)
                    copy_eng(oc * NHC + hc, ot[:, oc, 512 * hc:512 * (hc + 1)], om)
            nc.scalar.dma_start(out=d_out[j, :, :, :], in_=ot)

    nc.compile()
    return nc


def _prep_inputs(x, w1, b1, w2, b2, base_filters):
    """Host-side input layout prep. Returns per-core in_maps."""
    B = x.shape[0]
    # [B, 256, HW] -> [B, 2, 128, HW] -> [B, 128, 2, HW] so each
    # partition's DMA read is one contiguous 16KB block
    xs = (x.reshape(B, CCH, 128, HW).transpose(0, 2, 1, 3)
          .astype(ml_dtypes.bfloat16))
    xs = np.ascontiguousarray(xs)
    w1t = (np.ascontiguousarray(w1.T).astype(np.float32) / float(HW)
           ).reshape(CCH, 128, HID).transpose(1, 0, 2)
    w1t = np.ascontiguousarray(w1t)
    b1c = np.ascontiguousarray(b1.reshape(HID, 1)).astype(np.float32)
    w2p = np.ascontiguousarray(w2.reshape(CO, NB, HID).transpose(2, 1, 0)).astype(np.float32)
    b2t = np.ascontiguousarray(b2.reshape(2, 128, NB).transpose(1, 0, 2)).astype(np.float32)
    filt = base_filters.reshape(NB, CCH, 128, 3, 3)  # [n, cc, cp, dy, dx]
    # ft[c_part, cc, TAP_ROW[dy,dx] + n] = filt[n, cc, c_part, dy, dx]; gaps zero
    ft = np.zeros((128, CCH, 128), dtype=np.float32)
    for (dy, dx), r in TAP_ROW.items():
        ft[:, :, r:r + NB] = filt[:, :, :, dy, dx].transpose(2, 1, 0)
    ft = ft.astype(ml_dtypes.bfloat16)
    ident = np.eye(128, dtype=np.float32)
    zeros = np.zeros((128, 512), dtype=ml_dtypes.bfloat16)

    in_maps = []
    for core in range(N_CORES):
        in_maps.append({
            "x": np.ascontiguousarray(xs[core * BPC:(core + 1) * BPC]),
            "w1t": w1t, "b1": b1c, "w2p": w2p, "b2t": b2t,
            "ft": ft, "ident": ident, "zeros": zeros,
        })
    return in_maps


def kernel(x, w1, b1, w2, b2, base_filters):
    global _BUILT
    if _BUILT is None:
        _BUILT = _build()
    nc = _BUILT
    in_maps = _prep_inputs(np.asarray(x, dtype=np.float32),
                           np.asarray(w1, dtype=np.float32),
                           np.asarray(b1, dtype=np.float32),
                           np.asarray(w2, dtype=np.float32),
                           np.asarray(b2, dtype=np.float32),
                           np.asarray(base_filters, dtype=np.float32))
    res = run_bass_kernel_spmd(nc, in_maps, core_ids=list(range(N_CORES)))
    outs = []
    for core in range(N_CORES):
        o = np.asarray(res.results[core]["out"])   # [BPC, 128, 2, HW] bf16
        o = o.astype(np.float32).transpose(0, 2, 1, 3).reshape(BPC, CO, H, W)
        outs.append(o)
    return np.concatenate(outs, axis=0).astype(np.float32)


# revision 14
# speedup vs baseline: 1.4494x; 1.2320x over previous
"""DFMConv2d Trainium2 kernel (bf16 data path, v3).

Reference computation (per sample b):
  pooled = mean_{h,w} x[b]                          [C=256]
  h      = relu(pooled @ w1.T + b1)                 [128]
  mix    = softmax((h @ w2.T + b2).reshape(256, 8)) [256, 8]
  y      = conv3x3_SAME(x[b], base_filters)         [8, 64, 64]
  out[b] = einsum('on,nhw->ohw', mix, y)            [256, 64, 64]

8 NeuronCores, data-parallel over batch, 8 samples/core.  All bulk data
is bf16 (x, filters, conv output, mix weights, out) — the 2e-2 rel-err
budget tolerates ~5e-3 and it halves every byte moved.  MLP/softmax fp32.

  conv:  all 9 taps in the stationary M dim (tap slots spread over
         partitions 0..119), so x streams through the PE exactly twice;
         each cc chunk lands in its own half of one bf16 PSUM bank and a
         single tensor_tensor add drains+sums into ypad.
  shift: one SBUF->SBUF DMA per tap reading ypad at offset dy*64+dx.
         z is stored as [120, 2, 2050] (two 2048-col blocks + pad) so
         each 8-row shift emits 16 descriptors -> all 16 DMA engines.
  mix:   out[o, hw] = mixT.T @ z, K=120, single bf16-PSUM matmuls.
  MLP:   batched over sample pairs (N=2 moving) to amortize LDWEIGHTS.
  Rings: x loads on GPSIMD/SWDGE, z shifts on the SP ring, out stores +
  mixT replication on the ACT ring.  PSUM drains alternate DVE/ACT;
  GPSIMD does no compute (its per-op overhead is ~1us).
"""
import sys

sys.path.insert(0, "/opt/trn_rl_repo")

import numpy as np
import ml_dtypes

import concourse.bass as bass
import concourse.bacc as bacc
import concourse.tile as tile
import concourse.mybir as mybir
from concourse.bass_utils import run_bass_kernel_spmd
from contextlib import ExitStack

F32 = mybir.dt.float32
BF16 = mybir.dt.bfloat16
AFT = mybir.ActivationFunctionType
AXX = mybir.AxisListType.X
ALU = mybir.AluOpType

N_CORES = 8
BPC = 8            # samples per core
C = 256
CO = 256
H = W = 64
HW = H * W
NB = 8             # n_base
HID = 128
CCH = 2            # channel chunks of 128
NHC = 8            # h-chunks (8 output rows each)
M120 = 120         # tap rows live in partitions 0..119 (with gaps)
YP_LEN = 65 + 64 * 64 + 65   # lead halo + 64 rows + tail halo
ZB = 2050          # z block length: 2048 data + 2 pad (16 descs/shift)
# tap slots: dx=0 contiguous at base 0, dx=2 contiguous at base 96 (the
# column fixups need 32-aligned compute-op bases), dx=1 split across
TAP_ROW = {(0, 0): 0, (1, 0): 8, (2, 0): 16,
           (0, 1): 24, (1, 1): 32, (2, 1): 64,
           (0, 2): 96, (1, 2): 104, (2, 2): 112}

_BUILT = None


def _build():
    nc = bacc.Bacc("TRN2", target_bir_lowering=False)

    d_x = nc.dram_tensor("x", [BPC, 128, CCH, HW], BF16, kind="ExternalInput")
    d_w1t = nc.dram_tensor("w1t", [128, CCH, HID], F32, kind="ExternalInput")
    d_b1 = nc.dram_tensor("b1", [HID, 1], F32, kind="ExternalInput")
    d_w2p = nc.dram_tensor("w2p", [HID, NB, CO], F32, kind="ExternalInput")
    d_b2t = nc.dram_tensor("b2t", [128, 2, NB], F32, kind="ExternalInput")
    d_ft = nc.dram_tensor("ft", [128, CCH, 128], BF16, kind="ExternalInput")
    d_id = nc.dram_tensor("ident", [128, 128], F32, kind="ExternalInput")
    d_z0 = nc.dram_tensor("zeros", [128, 512], BF16, kind="ExternalInput")
    d_out = nc.dram_tensor("out", [BPC, 128, 2, HW], BF16, kind="ExternalOutput")

    with tile.TileContext(nc) as tc, ExitStack() as ctx:
        prm = ctx.enter_context(tc.tile_pool(name="prm", bufs=1))
        xp = ctx.enter_context(tc.tile_pool(name="xp", bufs=3))
        ypp = ctx.enter_context(tc.tile_pool(name="ypp", bufs=2))
        zp = ctx.enter_context(tc.tile_pool(name="zp", bufs=2))
        op = ctx.enter_context(tc.tile_pool(name="op", bufs=3))
        sm = ctx.enter_context(tc.tile_pool(name="sm", bufs=2))
        ps_c = ctx.enter_context(tc.tile_pool(name="ps_c", bufs=2, space="PSUM"))
        ps_m = ctx.enter_context(tc.tile_pool(name="ps_m", bufs=2, space="PSUM"))
        ps_s = ctx.enter_context(tc.tile_pool(name="ps_s", bufs=1, space="PSUM"))

        # ---- params (loaded once) ----
        w1t_sb = prm.tile([128, CCH, HID], F32, tag="w1t")
        nc.sync.dma_start(out=w1t_sb, in_=d_w1t[:, :, :])
        b1_sb = prm.tile([128, 1], F32, tag="b1")
        nc.sync.dma_start(out=b1_sb, in_=d_b1[:, :])
        w2p_sb = prm.tile([HID, NB, CO], F32, tag="w2p")
        nc.sync.dma_start(out=w2p_sb, in_=d_w2p[:, :, :])
        b2t_sb = prm.tile([128, 2, NB], F32, tag="b2t")
        nc.sync.dma_start(out=b2t_sb, in_=d_b2t[:, :, :])
        ft_sb = prm.tile([128, CCH, 128], BF16, tag="ft")
        nc.sync.dma_start(out=ft_sb, in_=d_ft[:, :, :])
        id_sb = prm.tile([128, 128], F32, tag="ident")
        nc.sync.dma_start(out=id_sb, in_=d_id[:, :])
        z0_sb = prm.tile([128, 512], BF16, tag="z0")
        nc.sync.dma_start(out=z0_sb, in_=d_z0[:, :])
        pooled_sb = prm.tile([128, CCH, BPC], F32, tag="pooled")
        h_sb = prm.tile([128, BPC], F32, tag="h")

        zts = {}
        for j in range(BPC):
            # ---- load (SWDGE ring) + pooling (DVE, per-cc for 2x mode) ----
            xt = xp.tile([128, CCH, HW], BF16, tag="x")
            nc.gpsimd.dma_start(out=xt, in_=d_x[j, :, :, :])
            for cc in range(CCH):
                nc.vector.reduce_sum(
                    pooled_sb[:, cc, j:j + 1], xt[:, cc, :], axis=AXX)

            # ---- conv into row-padded flat y_tap ----
            ypad = ypp.tile([M120, YP_LEN], BF16, tag="ypad")
            nc.scalar.copy(out=ypad[:, 0:65], in_=z0_sb[0:M120, 0:65])
            nc.scalar.copy(out=ypad[:, 65 + 4096:65 + 4096 + 65],
                           in_=z0_sb[0:M120, 0:65])
            for hc in range(NHC):
                yps = ps_c.tile([128, 512], F32, tag="yps")
                for cc in range(CCH):
                    nc.tensor.matmul(yps, ft_sb[:, cc, :],
                                     xt[:, cc, 512 * hc:512 * (hc + 1)],
                                     start=(cc == 0), stop=(cc == 1))
                dst = ypad[:, 65 + 512 * hc:65 + 512 * (hc + 1)]
                if hc % 2 == 0:
                    nc.vector.tensor_copy(dst, yps[0:M120, :])
                else:
                    nc.scalar.copy(out=dst, in_=yps[0:M120, :])

            # ---- per-tap shifted windows into z ----
            zt = zp.tile([M120, 2, ZB], BF16, tag="z")
            zts[j] = zt
            if j < 2:
                # one-time finite init of the gap rows (never copied; they
                # multiply against zero mix weights, but must not be NaN)
                zflat = zt.rearrange("p b c -> p (b c)")
                for r0 in (32, 64):
                    for s in range(8):
                        nc.vector.tensor_copy(
                            zflat[r0:r0 + 32, 512 * s:512 * (s + 1)],
                            z0_sb[r0:r0 + 32, 0:512])
                    nc.vector.tensor_copy(zflat[r0:r0 + 32, 4096:4100],
                                          z0_sb[r0:r0 + 32, 0:4])
            for (dy, dx), r in TAP_ROW.items():
                off = dy * 64 + dx
                nc.sync.dma_start(
                    out=zt[r:r + 8, :, 0:2048],
                    in_=ypad[r:r + 8, off:off + HW].rearrange(
                        "p (b c) -> p b c", b=2))
            # zero the dx wraparound columns: col 0 for dx=0 (rows 0:24),
            # col 63 for dx=2 (rows 96:120)
            for b in range(2):
                nc.vector.tensor_copy(
                    zt[0:24, b, 0:2048].rearrange("p (h w) -> p h w", w=64)
                    [:, :, 0:1].rearrange("p h w -> p (h w)"),
                    z0_sb[0:24, 0:32])
                nc.vector.tensor_copy(
                    zt[96:120, b, 0:2048].rearrange("p (h w) -> p h w", w=64)
                    [:, :, 63:64].rearrange("p h w -> p (h w)"),
                    z0_sb[96:120, 0:32])

            if j % 2 == 0:
                continue

            # ---- attention MLP for the pair (j-1, j) (fp32) ----
            # ph / pl2 / ptr all carved from one PSUM bank
            mlpb = ps_s.tile([128, 512], F32, tag="mlpb")
            ph = mlpb[:, 0:2]
            for cc in range(CCH):
                nc.tensor.matmul(ph, w1t_sb[:, cc, :],
                                 pooled_sb[:, cc, j - 1:j + 1],
                                 start=(cc == 0), stop=(cc == 1))
            nc.scalar.activation(out=h_sb[:, j - 1:j + 1], in_=ph,
                                 func=AFT.Relu, bias=b1_sb, scale=1.0)
            # [p, jj, oc, n]
            pl2 = mlpb[:, 64:96].rearrange("p (a b c) -> p a b c", a=2, b=2)
            for oc in range(2):
                for n in range(NB):
                    nc.tensor.matmul(pl2[:, :, oc, n],
                                     w2p_sb[:, n, oc * 128:(oc + 1) * 128],
                                     h_sb[:, j - 1:j + 1],
                                     start=True, stop=True)

            for jj in range(2):
                js = j - 1 + jj
                # ---- softmax over n (fp32) ----
                lg_sb = sm.tile([128, 2, NB], F32, tag="lg_sb")
                nc.vector.tensor_tensor(out=lg_sb, in0=pl2[:, jj], in1=b2t_sb,
                                        op=ALU.add)
                ex_sb = sm.tile([128, 2, NB], F32, tag="ex_sb")
                nc.scalar.activation(out=ex_sb.rearrange("p a b -> p (a b)"),
                                     in_=lg_sb.rearrange("p a b -> p (a b)"),
                                     func=AFT.Exp)
                sums = sm.tile([128, 2], F32, tag="sums")
                nc.vector.reduce_sum(sums, ex_sb, axis=AXX)
                rec = sm.tile([128, 2], F32, tag="rec")
                nc.vector.reciprocal(rec, sums)
                mixT_sb = sm.tile([M120, 2, 128], BF16, tag="mixT")
                nc.vector.tensor_copy(mixT_sb.rearrange("p a b -> p (a b)"),
                                      z0_sb[0:M120, 0:256])
                for oc in range(2):
                    mix_sb = sm.tile([128, NB], F32, tag="mix_sb")
                    nc.vector.tensor_scalar_mul(out=mix_sb, in0=ex_sb[:, oc, :],
                                                scalar1=rec[:, oc:oc + 1])
                    ptr = mlpb[0:NB, 128 + 128 * oc:256 + 128 * oc]
                    nc.tensor.transpose(ptr, mix_sb, id_sb)
                    nc.vector.tensor_copy(mixT_sb[0:NB, oc, :], ptr)
                # replicate seed rows [0:8) to the 9 tap slots (src and dst
                # ranges of each DMA must be disjoint)
                nc.scalar.dma_start(out=mixT_sb[8:16], in_=mixT_sb[0:8])
                nc.scalar.dma_start(out=mixT_sb[16:32], in_=mixT_sb[0:16])
                nc.scalar.dma_start(out=mixT_sb[32:40], in_=mixT_sb[0:8])
                nc.scalar.dma_start(out=mixT_sb[64:72], in_=mixT_sb[0:8])
                nc.scalar.dma_start(out=mixT_sb[96:120], in_=mixT_sb[0:24])

                # ---- mix: out[o, hw] = mixT.T @ z (K=120, bf16) ----
                ztj = zts[js]
                ot = op.tile([128, 2, HW], BF16, tag="out")
                for oc in range(2):
                    for q in range(4):
                        om2 = ps_m.tile([128, 1024], F32, tag="om2")
                        for k in range(2):
                            hc = 2 * q + k
                            nc.tensor.matmul(om2[:, 512 * k:512 * (k + 1)],
                                             mixT_sb[:, oc, :],
                                             ztj[:, hc // 4, 512 * (hc % 4):
                                                 512 * (hc % 4 + 1)],
                                             start=True, stop=True)
                        dst = ot[:, oc, 1024 * q:1024 * (q + 1)]
                        if (oc * 4 + q) % 2 == 0:
                            nc.vector.tensor_copy(dst, om2)
                        else:
                            nc.scalar.copy(out=dst, in_=om2)
                nc.scalar.dma_start(out=d_out[js, :, :, :], in_=ot)

    nc.compile()
    return nc


def _prep_inputs(x, w1, b1, w2, b2, base_filters):
    """Host-side input layout prep. Returns per-core in_maps."""
    B = x.shape[0]
    # [B, 256, HW] -> [B, 2, 128, HW] -> [B, 128, 2, HW] so each
    # partition's DMA read is one contiguous 16KB block
    xs = (x.reshape(B, CCH, 128, HW).transpose(0, 2, 1, 3)
          .astype(ml_dtypes.bfloat16))
    xs = np.ascontiguousarray(xs)
    w1t = (np.ascontiguousarray(w1.T).astype(np.float32) / float(HW)
           ).reshape(CCH, 128, HID).transpose(1, 0, 2)
    w1t = np.ascontiguousarray(w1t)
    b1c = np.ascontiguousarray(b1.reshape(HID, 1)).astype(np.float32)
    w2p = np.ascontiguousarray(w2.reshape(CO, NB, HID).transpose(2, 1, 0)).astype(np.float32)
    b2t = np.ascontiguousarray(b2.reshape(2, 128, NB).transpose(1, 0, 2)).astype(np.float32)
    filt = base_filters.reshape(NB, CCH, 128, 3, 3)  # [n, cc, cp, dy, dx]
    # ft[c_part, cc, TAP_ROW[dy,dx] + n] = filt[n, cc, c_part, dy, dx]; gaps zero
    ft = np.zeros((128, CCH, 128), dtype=np.float32)
    for (dy, dx), r in TAP_ROW.items():
        ft[:, :, r:r + NB] = filt[:, :, :, dy, dx].transpose(2, 1, 0)
    ft = ft.astype(ml_dtypes.bfloat16)
    ident = np.eye(128, dtype=np.float32)
    zeros = np.zeros((128, 512), dtype=ml_dtypes.bfloat16)

    in_maps = []
    for core in range(N_CORES):
        in_maps.append({
            "x": np.ascontiguousarray(xs[core * BPC:(core + 1) * BPC]),
            "w1t": w1t, "b1": b1c, "w2p": w2p, "b2t": b2t,
            "ft": ft, "ident": ident, "zeros": zeros,
        })
    return in_maps


def kernel(x, w1, b1, w2, b2, base_filters):
    global _BUILT
    if _BUILT is None:
        _BUILT = _build()
    nc = _BUILT
    in_maps = _prep_inputs(np.asarray(x, dtype=np.float32),
                           np.asarray(w1, dtype=np.float32),
                           np.asarray(b1, dtype=np.float32),
                           np.asarray(w2, dtype=np.float32),
                           np.asarray(b2, dtype=np.float32),
                           np.asarray(base_filters, dtype=np.float32))
    res = run_bass_kernel_spmd(nc, in_maps, core_ids=list(range(N_CORES)))
    outs = []
    for core in range(N_CORES):
        o = np.asarray(res.results[core]["out"])   # [BPC, 128, 2, HW] bf16
        o = o.astype(np.float32).transpose(0, 2, 1, 3).reshape(BPC, CO, H, W)
        outs.append(o)
    return np.concatenate(outs, axis=0).astype(np.float32)
